# revision 13
# baseline (speedup 1.0000x reference)
"""Trainium2 Bass kernel for a cross-modal transformer block (CrossModalBlock).

kernel(**inputs) takes the FULL unsharded inputs (numpy fp32) and returns the
full outputs (fuse, structure_embed, w1, w2) matching the reference.

Sharding (8 NeuronCores): data-parallel. Core c handles batch b=c//2, query
token half p=c%2 (512 of 1024 tokens). attn1 K/V come from structure_embed
(replicated per pair); attn2 K/V need post-LN2 activations of all 1024 tokens,
exchanged with a 1 MB pair-AllGather. No other collectives.

Device layout: feature-major activations [feature(partition), token(free)].
All matmuls run as float32r (full-rate). Attention scores are computed
transposed [k, q]; softmax denominators come from an extra ones-column in the
V operand of the probs@V matmul. Raw scores are written to DRAM as [h, k, q]
and transposed to [h, q, k] on the host during unsharding.
"""

import os
import sys

import numpy as np

sys.path.insert(0, "/opt/trn_rl_repo")
os.environ.setdefault("MYCRO_LOCAL_CACHE", "1")

import concourse.bass as bass  # noqa: E402
from concourse import bacc  # noqa: E402
import concourse.tile as tile  # noqa: E402
from concourse import mybir  # noqa: E402
from concourse.bass_utils import run_bass_kernel_spmd  # noqa: E402

B, L, SEQ, STR, H, DH, FF = 4, 1024, 1280, 512, 8, 64, 2048
EPS = 1e-5
NCORES = 8
TQ = 512          # query tokens per core
TK = 1024         # kv tokens (full batch)
F32 = mybir.dt.float32
F32R = mybir.dt.float32r
AF = mybir.ActivationFunctionType
OP = mybir.AluOpType

# bias table column offsets (each col holds 128 features)
BCOL = {}
_off = 0
for _name, _n in [
    ("p_b", 4), ("a1_qb", 4), ("a1_kb", 4), ("a1_ob", 4),
    ("f1_b", 16), ("f2_b", 4),
    ("a2_qb", 4), ("a2_kb", 4), ("a2_ob", 4),
    ("f3_b", 16), ("f4_b", 4),
]:
    BCOL[_name] = _off
    _off += _n
NBCOL = _off

TRACE = False
TRACE_ALL_CORES = False
LAST_RESULTS = None


def _install_ntff_hook():
    """This image's antenv lacks axon_hooks; synthesize it so trace=True works
    (mirrors trn_agent_boot._ntff_profile_via_ctypes). TRACE-only path."""
    import contextlib
    import ctypes
    import types

    try:
        from antenv.axon_hooks import get_axon_ntff_profile_hook  # noqa: F401
        return
    except ImportError:
        pass

    so_path = "/opt/axon/libaxon_pjrt.so"
    lib = ctypes.CDLL(so_path)
    if not hasattr(lib, "axon_start_nrt_profile"):
        hook = None
    else:
        lib.axon_start_nrt_profile.argtypes = [
            ctypes.POINTER(ctypes.c_int64), ctypes.c_size_t]
        lib.axon_start_nrt_profile.restype = ctypes.c_int64
        lib.axon_stop_nrt_profile.argtypes = [ctypes.c_char_p]
        lib.axon_stop_nrt_profile.restype = ctypes.c_int64

        @contextlib.contextmanager
        def hook(output_dir, device_ids):
            import jax

            jax.devices()
            if device_ids:
                ids = (ctypes.c_int64 * len(device_ids))(*device_ids)
                rc = lib.axon_start_nrt_profile(ids, len(device_ids))
            else:
                rc = lib.axon_start_nrt_profile(None, 0)
            if rc != 0:
                raise RuntimeError(f"axon_start_nrt_profile rc={rc}")
            try:
                yield
            finally:
                n = lib.axon_stop_nrt_profile(str(output_dir).encode())
                print(f"profile: {n} file(s) written to {output_dir}")

    mod = types.ModuleType("antenv.axon_hooks")
    mod.get_axon_ntff_profile_hook = lambda: hook
    mod.set_axon_ntff_profile_hook = lambda h: None
    sys.modules["antenv.axon_hooks"] = mod


def _emit(tc, t):
    nc = tc.nc

    S = tc.alloc_tile_pool(name="S", bufs=1)
    P = tc.alloc_tile_pool(name="P", bufs=2, space="PSUM")

    def mm(out, lhsT, rhs, start, stop):
        nc.tensor.matmul(out, lhsT.bitcast(F32R), rhs.bitcast(F32R),
                         start=start, stop=stop)

    # ---------------- constants ----------------
    onesf = S.tile([128, 8], F32, tag="onesf")
    nc.gpsimd.memset(onesf[:], 1.0)
    ones = S.tile([128, 1], F32R, tag="ones")
    nc.vector.tensor_copy(ones[:], onesf[:, 0:1])
    epsb = S.tile([1, 1], F32, tag="epsb")
    nc.gpsimd.memset(epsb[:], EPS)
    bias = S.tile([128, NBCOL], F32, tag="bias")
    nc.sync.dma_start(bias[:], t["bias"].ap())

    def bcol(name, i):
        return bias[:, BCOL[name] + i: BCOL[name] + i + 1]

    # ---------------- streamed weight/input panels (tag "w") ----------------
    def load_panel(name, nm, chunk_lo, chunk_hi, ncols):
        w = S.tile([128, (chunk_hi - chunk_lo) * ncols], F32R, tag="w", bufs=2,
                   name=nm)
        nc.sync.dma_start(
            w[:],
            t[name].ap().rearrange("(c p) n -> p c n", p=128)[:, chunk_lo:chunk_hi, :],
        )
        return w

    sT = S.tile([128, 4 * TK], F32R, tag="sT")  # structure.T [512,1024] resident
    nc.sync.dma_start(sT[:], t["sT"].ap().rearrange("(c p) n -> p c n", p=128))

    # =========================================================
    # proj_seq: x = seq @ p_w.T + p_b   (feature-major, my 512 tokens)
    # panel-outer accumulation, 4 psum halves in 2 "sc" tiles
    # =========================================================
    ps_proj = [P.tile([128, 1024], F32, tag="sc", name=f"psproj{i}")
               for i in range(2)]

    def proj_half(m):
        return ps_proj[m // 2][:, (m % 2) * 512:(m % 2) * 512 + 512]

    for p in range(2):
        pw_p = load_panel("pwT", f"pw{p}", 5 * p, 5 * p + 5, 512)
        x_p = load_panel("xT", f"xp{p}", 5 * p, 5 * p + 5, 512)
        for kcl in range(5):
            kc = 5 * p + kcl
            for m in range(4):
                mm(proj_half(m),
                   pw_p[:, kcl * 512 + m * 128: kcl * 512 + m * 128 + 128],
                   x_p[:, kcl * 512:(kcl + 1) * 512],
                   start=(kc == 0), stop=(kc == 9))
    x_tiles = []
    for m in range(4):
        xt = S.tile([128, 512], F32R, tag=f"x{m}", name=f"x{m}")
        nc.vector.tensor_scalar_add(xt[:], proj_half(m), bcol("p_b", m))
        x_tiles.append(xt)

    # =========================================================
    # attention (shared for attn1 / attn2)
    # =========================================================
    def attention(awA, awB, pre, q_src, kv_srcs, w_out, resid_tiles, out_tag):
        """q_src: 4 chunk APs [128, TQ]. kv_srcs: list of 2 slabs, each 4 chunk
        APs [128, 512]. Returns 4 tiles [128, TQ] = resid + attn_out + ob'."""

        def aw(kc, off, width=128):
            w = awA if kc < 2 else awB
            return w[:, (kc % 2) * 2048 + off: (kc % 2) * 2048 + off + width]

        # ---- q projection (+qb') ----
        q_tiles = []
        for m in range(4):
            ps = P.tile([128, 512], F32, tag="mm", name=f"{pre}qp{m}")
            for kc in range(4):
                mm(ps[:], aw(kc, m * 128), q_src[kc][:, :],
                   start=(kc == 0), stop=(kc == 3))
            qt = S.tile([128, 512], F32R, tag=f"atq{m}", name=f"{pre}q{m}")
            nc.vector.tensor_scalar_add(qt[:], ps[:], bcol(f"{pre}_qb", m))
            q_tiles.append(qt)

        # ---- k projection (+kb), all TK tokens ----
        k_tiles = []
        for m in range(4):
            kt = S.tile([128, TK], F32R, tag=f"atk{m}", name=f"{pre}k{m}")
            for s in range(2):
                ps = P.tile([128, 512], F32, tag="mm", name=f"{pre}kp{m}{s}")
                for kc in range(4):
                    mm(ps[:], aw(kc, 512 + m * 128), kv_srcs[s][kc][:, :],
                       start=(kc == 0), stop=(kc == 3))
                nc.vector.tensor_scalar_add(
                    kt[:, s * 512:(s + 1) * 512], ps[:], bcol(f"{pre}_kb", m))
            k_tiles.append(kt)

        # ---- v projection, token-major [tok, feat] + ones columns ----
        v_tiles = []
        for s in range(2):
            for tt in range(4):
                i = s * 4 + tt
                ps = P.tile([128, 512], F32, tag="mm", name=f"{pre}vp{i}")
                for kc in range(4):
                    mm(ps[:], kv_srcs[s][kc][:, tt * 128: tt * 128 + 128],
                       aw(kc, 1024, 512), start=(kc == 0), stop=(kc == 3))
                vt = S.tile([128, 8 * 65], F32R, tag=f"atv{i}", name=f"{pre}v{i}")
                v3 = vt.rearrange("p (h c) -> p h c", c=65)
                nc.vector.tensor_copy(
                    v3[:, :, 64:65],
                    onesf[:, 0:8].rearrange("p (a b) -> p a b", b=1))
                nc.vector.tensor_copy(
                    v3[:, :, 0:64], ps.rearrange("p (h c) -> p h c", c=64))
                v_tiles.append(vt)

        # ---- per head: scores^T -> raw dma + exp -> probs@V -> normalize ----
        ctx_tiles = [S.tile([128, 512], F32R, tag=f"atcf{i}", name=f"{pre}cf{i}")
                     for i in range(4)]
        for h in range(H):
            pt, off = h // 2, 64 * (h & 1)
            q_h = q_tiles[pt][off:off + 64, :]
            cps = P.tile([128, 512], F32, tag="cx", name=f"{pre}cps{h}")
            for j in range(4):
                sps = P.tile([128, 1024], F32, tag="sc", name=f"{pre}sps{h}{j}")
                for j2 in range(2):
                    ki = 2 * j + j2
                    mm(sps[:, j2 * 512:(j2 + 1) * 512],
                       k_tiles[pt][off:off + 64, ki * 128: ki * 128 + 128],
                       q_h, start=True, stop=True)
                dst = w_out.ap()[h, j * 256:(j + 1) * 256, :].rearrange(
                    "(j2 kp) q -> kp j2 q", kp=128)
                raw = S.tile([128, 1024], F32, tag="raw", bufs=2,
                             name=f"{pre}w{h}{j}")
                nc.vector.tensor_copy(raw[:], sps[:, :])
                nc.sync.dma_start(dst, raw.rearrange("p (j2 q) -> p j2 q", j2=2))
                et = S.tile([128, 1024], F32R, tag="expT", bufs=2,
                            name=f"{pre}e{h}{j}")
                nc.scalar.activation(et[:], sps[:], AF.Exp)
                for j2 in range(2):
                    ki = 2 * j + j2
                    mm(cps[0:65, :], v_tiles[ki][:, h * 65: h * 65 + 65],
                       et[:, j2 * 512:(j2 + 1) * 512],
                       start=(ki == 0), stop=(ki == 7))
            rec = S.tile([1, 512], F32, tag="sm", bufs=7, name=f"{pre}r{h}")
            nc.vector.reciprocal(rec[:], cps[64:65, :])
            rbc = S.tile([64, 512], F32, tag="rbc", bufs=2, name=f"{pre}rb{h}")
            nc.gpsimd.partition_broadcast(rbc[:], rec[:])
            nc.vector.tensor_mul(ctx_tiles[pt][off:off + 64, :],
                                 cps[0:64, :], rbc[:])

        # ---- out projection + ob' + residual ----
        out_tiles = []
        for m in range(4):
            ps = P.tile([128, 512], F32, tag="mm", name=f"{pre}op{m}")
            for kc in range(4):
                mm(ps[:], aw(kc, 1536 + m * 128), ctx_tiles[kc][:, :],
                   start=(kc == 0), stop=(kc == 3))
            ot = S.tile([128, 512], F32R, tag=f"{out_tag}{m}", name=f"{pre}o{m}")
            nc.vector.scalar_tensor_tensor(
                ot[:], in0=ps[:], scalar=bcol(f"{pre}_ob", m),
                in1=resid_tiles[m][:, :].bitcast(F32), op0=OP.add, op1=OP.add)
            out_tiles.append(ot)
        return out_tiles

    # =========================================================
    # layernorm (feature-major over STR=512; g=1, b=0 in this model)
    # =========================================================
    def layernorm(x_tiles, out_tag):
        s1 = P.tile([128, 512], F32, tag="cx", name=f"{out_tag}s1")
        s2 = P.tile([128, 512], F32, tag="cx", name=f"{out_tag}s2")
        sq_tiles = []
        for c in range(4):
            sqt = S.tile([128, 512], F32R, tag="sq", bufs=2, name=f"{out_tag}sq{c}")
            xf = x_tiles[c][:, :].bitcast(F32)
            nc.vector.tensor_mul(sqt[:], xf, xf)
            sq_tiles.append(sqt)
        for c in range(4):
            mm(s1[0:1, :], ones[:, 0:1], x_tiles[c][:, :],
               start=(c == 0), stop=(c == 3))
        for c in range(4):
            mm(s2[0:1, :], ones[:, 0:1], sq_tiles[c][:],
               start=(c == 0), stop=(c == 3))

        def sm(nm):
            return S.tile([1, 512], F32, tag="sm", bufs=7, name=f"{out_tag}{nm}")

        mean = sm("mean")
        nc.vector.tensor_scalar_mul(mean[:], s1[0:1, :], 1.0 / STR)
        ms = sm("ms")
        nc.vector.tensor_scalar_mul(ms[:], s2[0:1, :], 1.0 / STR)
        m2 = sm("m2")
        nc.vector.tensor_mul(m2[:], mean[:], mean[:])
        var = sm("var")
        nc.vector.tensor_sub(var[:], ms[:], m2[:])
        std = sm("std")
        nc.scalar.activation(std[:], var[:], AF.Sqrt, bias=epsb[:], scale=1.0)
        rstd = sm("rstd")
        nc.vector.reciprocal(rstd[:], std[:])
        mr = sm("mr")
        nc.vector.tensor_mul(mr[:], mean[:], rstd[:])
        rstd_bc = S.tile([128, 512], F32, tag="bc", bufs=2, name=f"{out_tag}rb")
        nc.gpsimd.partition_broadcast(rstd_bc[:], rstd[:])
        mr_bc = S.tile([128, 512], F32, tag="bc", bufs=2, name=f"{out_tag}mb")
        nc.gpsimd.partition_broadcast(mr_bc[:], mr[:])
        out = []
        for c in range(4):
            otag = f"atcf{c}" if out_tag == "h2l" else f"{out_tag}{c}"
            o = S.tile([128, 512], F32R, tag=otag, name=f"{out_tag}{c}")
            nc.vector.tensor_mul(o[:], x_tiles[c][:, :].bitcast(F32), rstd_bc[:])
            nc.vector.tensor_sub(o[:], o[:].bitcast(F32), mr_bc[:])
            out.append(o)
        return out

    # =========================================================
    # ffn: resid + gelu(x@f1+b1)@f2 + b2
    # =========================================================
    def ffn(f1A, f1B, f2A, f2B, b1name, b2name, x_tiles, resid_tiles, out_tag):
        h_tiles = []
        for m in range(16):
            ps = P.tile([128, 512], F32, tag="mm", name=f"{out_tag}hp{m}")
            for kc in range(4):
                w = f1A if kc < 2 else f1B
                mm(ps[:], w[:, (kc % 2) * 2048 + m * 128: (kc % 2) * 2048 + m * 128 + 128],
                   x_tiles[kc][:, :], start=(kc == 0), stop=(kc == 3))
            ht = S.tile([128, 512], F32R, tag=f"ffnh{m}", name=f"{out_tag}h{m}")
            nc.scalar.activation(ht[:], ps[:], AF.Gelu, bias=bcol(b1name, m),
                                 scale=1.0)
            h_tiles.append(ht)
        out_tiles = []
        for m in range(4):
            ps = P.tile([128, 512], F32, tag="mm", name=f"{out_tag}op{m}")
            for hc in range(16):
                w = f2A if hc < 8 else f2B
                mm(ps[:], w[:, (hc % 8) * 512 + m * 128: (hc % 8) * 512 + m * 128 + 128],
                   h_tiles[hc][:], start=(hc == 0), stop=(hc == 15))
            ot = S.tile([128, 512], F32R, tag=f"{out_tag}{m}", name=f"{out_tag}o{m}")
            nc.vector.scalar_tensor_tensor(
                ot[:], in0=ps[:], scalar=bcol(b2name, m),
                in1=resid_tiles[m][:, :].bitcast(F32), op0=OP.add, op1=OP.add)
            out_tiles.append(ot)
        return out_tiles

    # =========================================================
    # the block
    # =========================================================
    a1A = load_panel("a1w", "a1A", 0, 2, 2048)
    a1B = load_panel("a1w", "a1B", 2, 4, 2048)
    s_slabs = [[sT[:, c * TK + s * 512: c * TK + (s + 1) * 512] for c in range(4)]
               for s in range(2)]
    fuse1 = attention(a1A, a1B, "a1", x_tiles, s_slabs, t["w1T"], x_tiles, "fuA")

    h1 = layernorm(fuse1, "hln")
    f1A = load_panel("f1wT", "f1A", 0, 2, 2048)
    f1B = load_panel("f1wT", "f1B", 2, 4, 2048)
    f2A = load_panel("f2wT", "f2A", 0, 8, 512)
    f2B = load_panel("f2wT", "f2B", 8, 16, 512)
    fuse2 = ffn(f1A, f1B, f2A, f2B, "f1_b", "f2_b", h1, fuse1, "fuB")

    # LN2 on my tokens, exchange within the pair for full-token K/V
    h2loc = layernorm(fuse2, "h2l")
    for c in range(4):
        nc.sync.dma_start(t["cc_in"].ap()[c * 128:(c + 1) * 128, :], h2loc[c][:])
    nc.gpsimd.collective_compute(
        "AllGather", OP.bypass,
        replica_groups=[[0, 1], [2, 3], [4, 5], [6, 7]],
        ins=[t["cc_in"].ap().opt()],
        outs=[t["cc_out"].ap().opt()],
    )
    h2_slabs = []
    for s in range(2):
        slab = []
        for c in range(4):
            tag = f"x{c}" if s == 0 else f"hln{c}"
            st = S.tile([128, 512], F32R, tag=tag, name=f"h2s{s}{c}")
            nc.sync.dma_start(
                st[:],
                t["cc_out"].ap()[s * 512 + c * 128: s * 512 + (c + 1) * 128, :])
            slab.append(st)
        h2_slabs.append(slab)

    a2A = load_panel("a2w", "a2A", 0, 2, 2048)
    a2B = load_panel("a2w", "a2B", 2, 4, 2048)
    fuse3 = attention(a2A, a2B, "a2", h2loc, h2_slabs, t["w2T"], fuse2, "fuA")

    h3 = layernorm(fuse3, "hln")
    f3A = load_panel("f3wT", "f3A", 0, 2, 2048)
    f3B = load_panel("f3wT", "f3B", 2, 4, 2048)
    f4A = load_panel("f4wT", "f4A", 0, 8, 512)
    f4B = load_panel("f4wT", "f4B", 8, 16, 512)
    out_t = ffn(f3A, f3B, f4A, f4B, "f3_b", "f4_b", h3, fuse3, "fuB")

    for c in range(4):
        nc.sync.dma_start(t["fuseT"].ap()[c * 128:(c + 1) * 128, :],
                          out_t[c][:].bitcast(F32))

    P.release()
    S.release()


def _build():
    nc = bacc.Bacc("TRN2", target_bir_lowering=False, debug=False,
                   num_devices=NCORES)
    t = {}
    t["xT"] = nc.dram_tensor("xT", [SEQ, TQ], F32R, kind="ExternalInput")
    t["sT"] = nc.dram_tensor("sT", [STR, TK], F32R, kind="ExternalInput")
    t["pwT"] = nc.dram_tensor("pwT", [SEQ, STR], F32R, kind="ExternalInput")
    for nm in ("a1w", "a2w", "f1wT", "f3wT"):
        t[nm] = nc.dram_tensor(nm, [STR, 4 * STR], F32R, kind="ExternalInput")
    for nm in ("f2wT", "f4wT"):
        t[nm] = nc.dram_tensor(nm, [FF, STR], F32R, kind="ExternalInput")
    t["bias"] = nc.dram_tensor("bias", [128, NBCOL], F32, kind="ExternalInput")
    t["w1T"] = nc.dram_tensor("w1T", [H, TK, TQ], F32, kind="ExternalOutput")
    t["w2T"] = nc.dram_tensor("w2T", [H, TK, TQ], F32, kind="ExternalOutput")
    t["fuseT"] = nc.dram_tensor("fuseT", [STR, TQ], F32, kind="ExternalOutput")
    t["cc_in"] = nc.dram_tensor("cc_in", [STR, TQ], F32R)
    t["cc_out"] = nc.dram_tensor("cc_out", [2 * STR, TQ], F32R)

    with tile.TileContext(nc) as tc:
        _emit(tc, t)
    nc.compile()
    return nc


def _pack_bias(d):
    scale = DH ** -0.5
    tbl = np.zeros((128, NBCOL), np.float32)

    def put(name, vec):
        n = vec.shape[0] // 128
        tbl[:, BCOL[name]:BCOL[name] + n] = vec.reshape(n, 128).T

    put("p_b", d["p_b"])
    for pre in ("a1", "a2"):
        put(f"{pre}_qb", d[f"{pre}_qb"] * scale)
        put(f"{pre}_kb", d[f"{pre}_kb"])
        # fold v-bias through out projection: ob' = ob + vb @ ow.T
        put(f"{pre}_ob", d[f"{pre}_ob"] + d[f"{pre}_vb"] @ d[f"{pre}_ow"].T)
    for nm in ("f1_b", "f2_b", "f3_b", "f4_b"):
        put(nm, d[nm])
    return tbl


def kernel(**inputs):
    global LAST_RESULTS
    d = {k: np.ascontiguousarray(np.asarray(v, dtype=np.float32))
         for k, v in inputs.items()}
    scale = DH ** -0.5

    shared = {
        "pwT": np.ascontiguousarray(d["p_w"].T),
        "f1wT": np.ascontiguousarray(d["f1_w"].T),
        "f2wT": np.ascontiguousarray(d["f2_w"].T),
        "f3wT": np.ascontiguousarray(d["f3_w"].T),
        "f4wT": np.ascontiguousarray(d["f4_w"].T),
        "bias": _pack_bias(d),
    }
    for pre in ("a1", "a2"):
        shared[f"{pre}w"] = np.ascontiguousarray(np.concatenate(
            [(d[f"{pre}_qw"] * scale).T, d[f"{pre}_kw"].T,
             d[f"{pre}_vw"].T, d[f"{pre}_ow"].T], axis=1))

    in_maps = []
    for c in range(NCORES):
        b, p = c // 2, c % 2
        m = dict(shared)
        m["xT"] = np.ascontiguousarray(d["seq_embed"][b].T[:, p * TQ:(p + 1) * TQ])
        m["sT"] = np.ascontiguousarray(d["structure_embed"][b].T)
        in_maps.append(m)

    if TRACE:
        _install_ntff_hook()
    nc = _build()
    res = run_bass_kernel_spmd(
        nc, in_maps, core_ids=list(range(NCORES)),
        trace=TRACE,
        trace_cores=(list(range(NCORES)) if TRACE_ALL_CORES else [0]) if TRACE
        else None,
    )
    LAST_RESULTS = res

    fuse = np.empty((B, L, STR), np.float32)
    w1 = np.empty((B * H, L, TK), np.float32)
    w2 = np.empty((B * H, L, TK), np.float32)
    for c in range(NCORES):
        b, p = c // 2, c % 2
        r = res.results[c]
        qs = slice(p * TQ, (p + 1) * TQ)
        fuse[b, qs, :] = r["fuseT"].T
        w1[b * H:(b + 1) * H, qs, :] = r["w1T"].transpose(0, 2, 1)
        w2[b * H:(b + 1) * H, qs, :] = r["w2T"].transpose(0, 2, 1)
    return fuse, d["structure_embed"], w1, w2


# revision 15
# speedup vs baseline: 1.0929x; 1.0929x over previous
"""Trainium2 Bass kernel for a cross-modal transformer block (CrossModalBlock).

kernel(**inputs) takes the FULL unsharded inputs (numpy fp32) and returns the
full outputs (fuse, structure_embed, w1, w2) matching the reference.

Sharding (8 NeuronCores): data-parallel. Core c handles batch b=c//2, query
token half p=c%2 (512 of 1024 tokens). attn1 K/V come from structure_embed
(replicated per pair); attn2 K/V need post-LN2 activations of all 1024 tokens,
exchanged with a 1 MB pair-AllGather. No other collectives.

Device layout: feature-major activations [feature(partition), token(free)].
All matmuls run as float32r (full-rate). Attention scores are computed
transposed [k, q]; softmax denominators come from an extra ones-column in the
V operand of the probs@V matmul. Raw scores are written to DRAM as [h, k, q]
and transposed to [h, q, k] on the host during unsharding.
"""

import os
import sys

import numpy as np

sys.path.insert(0, "/opt/trn_rl_repo")
os.environ.setdefault("MYCRO_LOCAL_CACHE", "1")

import concourse.bass as bass  # noqa: E402
from concourse import bacc  # noqa: E402
import concourse.tile as tile  # noqa: E402
from concourse import mybir  # noqa: E402
from concourse.bass_utils import run_bass_kernel_spmd  # noqa: E402

B, L, SEQ, STR, H, DH, FF = 4, 1024, 1280, 512, 8, 64, 2048
EPS = 1e-5
NCORES = 8
TQ = 512          # query tokens per core
TK = 1024         # kv tokens (full batch)
F32 = mybir.dt.float32
F32R = mybir.dt.float32r
AF = mybir.ActivationFunctionType
OP = mybir.AluOpType

# bias table column offsets (each col holds 128 features)
BCOL = {}
_off = 0
for _name, _n in [
    ("p_b", 4), ("a1_qb", 4), ("a1_kb", 4), ("a1_ob", 4),
    ("f1_b", 16), ("f2_b", 4),
    ("a2_qb", 4), ("a2_kb", 4), ("a2_ob", 4),
    ("f3_b", 16), ("f4_b", 4),
]:
    BCOL[_name] = _off
    _off += _n
NBCOL = _off

TRACE = False
TRACE_ALL_CORES = False
LAST_RESULTS = None


def _install_ntff_hook():
    """This image's antenv lacks axon_hooks; synthesize it so trace=True works
    (mirrors trn_agent_boot._ntff_profile_via_ctypes). TRACE-only path."""
    import contextlib
    import ctypes
    import types

    try:
        from antenv.axon_hooks import get_axon_ntff_profile_hook  # noqa: F401
        return
    except ImportError:
        pass

    so_path = "/opt/axon/libaxon_pjrt.so"
    lib = ctypes.CDLL(so_path)
    if not hasattr(lib, "axon_start_nrt_profile"):
        hook = None
    else:
        lib.axon_start_nrt_profile.argtypes = [
            ctypes.POINTER(ctypes.c_int64), ctypes.c_size_t]
        lib.axon_start_nrt_profile.restype = ctypes.c_int64
        lib.axon_stop_nrt_profile.argtypes = [ctypes.c_char_p]
        lib.axon_stop_nrt_profile.restype = ctypes.c_int64

        @contextlib.contextmanager
        def hook(output_dir, device_ids):
            import jax

            jax.devices()
            if device_ids:
                ids = (ctypes.c_int64 * len(device_ids))(*device_ids)
                rc = lib.axon_start_nrt_profile(ids, len(device_ids))
            else:
                rc = lib.axon_start_nrt_profile(None, 0)
            if rc != 0:
                raise RuntimeError(f"axon_start_nrt_profile rc={rc}")
            try:
                yield
            finally:
                n = lib.axon_stop_nrt_profile(str(output_dir).encode())
                print(f"profile: {n} file(s) written to {output_dir}")

    mod = types.ModuleType("antenv.axon_hooks")
    mod.get_axon_ntff_profile_hook = lambda: hook
    mod.set_axon_ntff_profile_hook = lambda h: None
    sys.modules["antenv.axon_hooks"] = mod


def _emit(tc, t):
    nc = tc.nc

    S = tc.alloc_tile_pool(name="S", bufs=1)
    P = tc.alloc_tile_pool(name="P", bufs=2, space="PSUM")

    def mm(out, lhsT, rhs, start, stop):
        nc.tensor.matmul(out, lhsT.bitcast(F32R), rhs.bitcast(F32R),
                         start=start, stop=stop)

    # ---------------- constants ----------------
    onesf = S.tile([128, 8], F32, tag="onesf")
    nc.gpsimd.memset(onesf[:], 1.0)
    ones = S.tile([128, 1], F32R, tag="ones")
    nc.vector.tensor_copy(ones[:], onesf[:, 0:1])
    epsb = S.tile([1, 1], F32, tag="epsb")
    nc.gpsimd.memset(epsb[:], EPS)
    bias = S.tile([128, NBCOL], F32, tag="bias")
    nc.sync.dma_start(bias[:], t["bias"].ap())

    def bcol(name, i):
        return bias[:, BCOL[name] + i: BCOL[name] + i + 1]

    # ---------------- streamed weight/input panels (tag "w") ----------------
    def load_panel(name, nm, chunk_lo, chunk_hi, ncols):
        w = S.tile([128, (chunk_hi - chunk_lo) * ncols], F32R, tag="w", bufs=2,
                   name=nm)
        nc.sync.dma_start(
            w[:],
            t[name].ap().rearrange("(c p) n -> p c n", p=128)[:, chunk_lo:chunk_hi, :],
        )
        return w

    # =========================================================
    # proj_seq: x = seq @ p_w.T + p_b   (feature-major, my 512 tokens)
    # panel-outer accumulation, 4 psum halves in 2 "sc" tiles
    # =========================================================
    ps_proj = [P.tile([128, 1024], F32, tag="sc", name=f"psproj{i}")
               for i in range(2)]

    def proj_half(m):
        return ps_proj[m // 2][:, (m % 2) * 512:(m % 2) * 512 + 512]

    for p in range(2):
        pw_p = load_panel("pwT", f"pw{p}", 5 * p, 5 * p + 5, 512)
        x_p = load_panel("xT", f"xp{p}", 5 * p, 5 * p + 5, 512)
        for kcl in range(5):
            kc = 5 * p + kcl
            for m in range(4):
                mm(proj_half(m),
                   pw_p[:, kcl * 512 + m * 128: kcl * 512 + m * 128 + 128],
                   x_p[:, kcl * 512:(kcl + 1) * 512],
                   start=(kc == 0), stop=(kc == 9))
    x_tiles = []
    for m in range(4):
        xt = S.tile([128, 512], F32R, tag=f"x{m}", name=f"x{m}")
        nc.vector.tensor_scalar_add(xt[:], proj_half(m), bcol("p_b", m))
        x_tiles.append(xt)

    sT = S.tile([128, 4 * TK], F32R, tag="sT")  # structure.T [512,1024] resident
    nc.sync.dma_start(sT[:], t["sT"].ap().rearrange("(c p) n -> p c n", p=128))

    # =========================================================
    # attention (shared for attn1 / attn2)
    # =========================================================
    def attention(awA, awB, pre, q_src, kv_srcs, w_out, resid_tiles, out_tag):
        """q_src: 4 chunk APs [128, TQ]. kv_srcs: list of 2 slabs, each 4 chunk
        APs [128, 512]. Returns 4 tiles [128, TQ] = resid + attn_out + ob'."""

        def aw(kc, off, width=128):
            w = awA if kc < 2 else awB
            return w[:, (kc % 2) * 2048 + off: (kc % 2) * 2048 + off + width]

        # ---- q projection (+qb') ----
        q_tiles = []
        for m in range(4):
            ps = P.tile([128, 512], F32, tag="mm", name=f"{pre}qp{m}")
            for kc in range(4):
                mm(ps[:], aw(kc, m * 128), q_src[kc][:, :],
                   start=(kc == 0), stop=(kc == 3))
            qt = S.tile([128, 512], F32R, tag=f"atq{m}", name=f"{pre}q{m}")
            nc.vector.tensor_scalar_add(qt[:], ps[:], bcol(f"{pre}_qb", m))
            q_tiles.append(qt)

        # ---- k projection (+kb), all TK tokens ----
        k_tiles = []
        for m in range(4):
            kt = S.tile([128, TK], F32R, tag=f"atk{m}", name=f"{pre}k{m}")
            for s in range(2):
                ps = P.tile([128, 512], F32, tag="mm", name=f"{pre}kp{m}{s}")
                for kc in range(4):
                    mm(ps[:], aw(kc, 512 + m * 128), kv_srcs[s][kc][:, :],
                       start=(kc == 0), stop=(kc == 3))
                nc.vector.tensor_scalar_add(
                    kt[:, s * 512:(s + 1) * 512], ps[:], bcol(f"{pre}_kb", m))
            k_tiles.append(kt)

        # ---- v projection, token-major [tok, feat] + ones columns ----
        v_tiles = []
        for s in range(2):
            for tt in range(4):
                i = s * 4 + tt
                ps = P.tile([128, 512], F32, tag="mm", name=f"{pre}vp{i}")
                for kc in range(4):
                    mm(ps[:], kv_srcs[s][kc][:, tt * 128: tt * 128 + 128],
                       aw(kc, 1024, 512), start=(kc == 0), stop=(kc == 3))
                vt = S.tile([128, 8 * 65], F32R, tag=f"atv{i}", name=f"{pre}v{i}")
                v3 = vt.rearrange("p (h c) -> p h c", c=65)
                nc.vector.tensor_copy(
                    v3[:, :, 64:65],
                    onesf[:, 0:8].rearrange("p (a b) -> p a b", b=1))
                nc.vector.tensor_copy(
                    v3[:, :, 0:64], ps.rearrange("p (h c) -> p h c", c=64))
                v_tiles.append(vt)

        # ---- per head: scores^T -> raw dma + exp -> probs@V -> normalize ----
        ctx_tiles = [S.tile([128, 512], F32R, tag=f"atcf{i}", name=f"{pre}cf{i}")
                     for i in range(4)]
        for h in range(H):
            pt, off = h // 2, 64 * (h & 1)
            q_h = q_tiles[pt][off:off + 64, :]
            cps = P.tile([128, 512], F32, tag="cx", name=f"{pre}cps{h}")
            for j in range(4):
                sps = P.tile([128, 1024], F32, tag="sc", name=f"{pre}sps{h}{j}")
                for j2 in range(2):
                    ki = 2 * j + j2
                    mm(sps[:, j2 * 512:(j2 + 1) * 512],
                       k_tiles[pt][off:off + 64, ki * 128: ki * 128 + 128],
                       q_h, start=True, stop=True)
                dst = w_out.ap()[h, j * 256:(j + 1) * 256, :].rearrange(
                    "(j2 kp) q -> kp j2 q", kp=128)
                raw = S.tile([128, 1024], F32, tag="raw", bufs=2,
                             name=f"{pre}w{h}{j}")
                if j % 2 == 0:
                    nc.vector.tensor_copy(raw[:], sps[:, :])
                else:
                    nc.scalar.activation(raw[:], sps[:, :], AF.Identity)
                nc.sync.dma_start(dst, raw.rearrange("p (j2 q) -> p j2 q", j2=2))
                et = S.tile([128, 1024], F32R, tag="expT", bufs=2,
                            name=f"{pre}e{h}{j}")
                nc.scalar.activation(et[:], sps[:], AF.Exp)
                for j2 in range(2):
                    ki = 2 * j + j2
                    mm(cps[0:65, :], v_tiles[ki][:, h * 65: h * 65 + 65],
                       et[:, j2 * 512:(j2 + 1) * 512],
                       start=(ki == 0), stop=(ki == 7))
            den = S.tile([1, 512], F32, tag="sm", bufs=7, name=f"{pre}d{h}")
            nc.vector.tensor_copy(den[:], cps[64:65, :])
            rec = S.tile([1, 512], F32, tag="sm", bufs=7, name=f"{pre}r{h}")
            rsc = S.tile([1, 512], F32, tag="rsc", bufs=2, name=f"{pre}rs{h}")
            nc.vector.reciprocal_approx_accurate(rec[:], den[:], rsc[:])
            rbc = S.tile([64, 512], F32, tag="rbc", bufs=2, name=f"{pre}rb{h}")
            nc.gpsimd.partition_broadcast(rbc[:], rec[:])
            nc.vector.tensor_mul(ctx_tiles[pt][off:off + 64, :],
                                 cps[0:64, :], rbc[:])

        # ---- out projection + ob' + residual ----
        out_tiles = []
        for m in range(4):
            ps = P.tile([128, 512], F32, tag="mm", name=f"{pre}op{m}")
            for kc in range(4):
                mm(ps[:], aw(kc, 1536 + m * 128), ctx_tiles[kc][:, :],
                   start=(kc == 0), stop=(kc == 3))
            ot = S.tile([128, 512], F32R, tag=f"{out_tag}{m}", name=f"{pre}o{m}")
            nc.vector.scalar_tensor_tensor(
                ot[:], in0=ps[:], scalar=bcol(f"{pre}_ob", m),
                in1=resid_tiles[m][:, :].bitcast(F32), op0=OP.add, op1=OP.add)
            out_tiles.append(ot)
        return out_tiles

    # =========================================================
    # layernorm (feature-major over STR=512; g=1, b=0 in this model)
    # =========================================================
    def layernorm(x_tiles, out_tag):
        s1 = P.tile([128, 512], F32, tag="cx", name=f"{out_tag}s1")
        s2 = P.tile([128, 512], F32, tag="cx", name=f"{out_tag}s2")
        sq_tiles = []
        for c in range(4):
            sqt = S.tile([128, 512], F32R, tag="sq", bufs=2, name=f"{out_tag}sq{c}")
            xf = x_tiles[c][:, :].bitcast(F32)
            nc.vector.tensor_mul(sqt[:], xf, xf)
            sq_tiles.append(sqt)
        for c in range(4):
            mm(s1[0:1, :], ones[:, 0:1], x_tiles[c][:, :],
               start=(c == 0), stop=(c == 3))
        for c in range(4):
            mm(s2[0:1, :], ones[:, 0:1], sq_tiles[c][:],
               start=(c == 0), stop=(c == 3))

        def sm(nm):
            return S.tile([1, 512], F32, tag="sm", bufs=7, name=f"{out_tag}{nm}")

        mean = sm("mean")
        nc.vector.tensor_scalar_mul(mean[:], s1[0:1, :], 1.0 / STR)
        ms = sm("ms")
        nc.vector.tensor_scalar_mul(ms[:], s2[0:1, :], 1.0 / STR)
        m2 = sm("m2")
        nc.vector.tensor_mul(m2[:], mean[:], mean[:])
        var = sm("var")
        nc.vector.tensor_sub(var[:], ms[:], m2[:])
        std = sm("std")
        nc.scalar.activation(std[:], var[:], AF.Sqrt, bias=epsb[:], scale=1.0)
        rstd = sm("rstd")
        rsc2 = S.tile([1, 512], F32, tag="rsc", bufs=2, name=f"{out_tag}rsc")
        nc.vector.reciprocal_approx_accurate(rstd[:], std[:], rsc2[:])
        mr = sm("mr")
        nc.vector.tensor_mul(mr[:], mean[:], rstd[:])
        rstd_bc = S.tile([128, 512], F32, tag="bc", bufs=2, name=f"{out_tag}rb")
        nc.gpsimd.partition_broadcast(rstd_bc[:], rstd[:])
        mr_bc = S.tile([128, 512], F32, tag="bc", bufs=2, name=f"{out_tag}mb")
        nc.gpsimd.partition_broadcast(mr_bc[:], mr[:])
        out = []
        for c in range(4):
            otag = f"atcf{c}" if out_tag == "h2l" else f"{out_tag}{c}"
            o = S.tile([128, 512], F32R, tag=otag, name=f"{out_tag}{c}")
            nc.vector.tensor_mul(o[:], x_tiles[c][:, :].bitcast(F32), rstd_bc[:])
            nc.vector.tensor_sub(o[:], o[:].bitcast(F32), mr_bc[:])
            out.append(o)
        return out

    # =========================================================
    # ffn: resid + gelu(x@f1+b1)@f2 + b2
    # =========================================================
    def ffn(f1A, f1B, f2A, f2B, b1name, b2name, x_tiles, resid_tiles, out_tag):
        h_tiles = []
        for m in range(16):
            ps = P.tile([128, 512], F32, tag="mm", name=f"{out_tag}hp{m}")
            for kc in range(4):
                w = f1A if kc < 2 else f1B
                mm(ps[:], w[:, (kc % 2) * 2048 + m * 128: (kc % 2) * 2048 + m * 128 + 128],
                   x_tiles[kc][:, :], start=(kc == 0), stop=(kc == 3))
            ht = S.tile([128, 512], F32R, tag=f"ffnh{m}", name=f"{out_tag}h{m}")
            nc.scalar.activation(ht[:], ps[:], AF.Gelu, bias=bcol(b1name, m),
                                 scale=1.0)
            h_tiles.append(ht)
        out_tiles = []
        for m in range(4):
            ps = P.tile([128, 512], F32, tag="mm", name=f"{out_tag}op{m}")
            for hc in range(16):
                w = f2A if hc < 8 else f2B
                mm(ps[:], w[:, (hc % 8) * 512 + m * 128: (hc % 8) * 512 + m * 128 + 128],
                   h_tiles[hc][:], start=(hc == 0), stop=(hc == 15))
            ot = S.tile([128, 512], F32R, tag=f"{out_tag}{m}", name=f"{out_tag}o{m}")
            nc.vector.scalar_tensor_tensor(
                ot[:], in0=ps[:], scalar=bcol(b2name, m),
                in1=resid_tiles[m][:, :].bitcast(F32), op0=OP.add, op1=OP.add)
            out_tiles.append(ot)
        return out_tiles

    # =========================================================
    # the block
    # =========================================================
    a1A = load_panel("a1w", "a1A", 0, 2, 2048)
    a1B = load_panel("a1w", "a1B", 2, 4, 2048)
    s_slabs = [[sT[:, c * TK + s * 512: c * TK + (s + 1) * 512] for c in range(4)]
               for s in range(2)]
    fuse1 = attention(a1A, a1B, "a1", x_tiles, s_slabs, t["w1T"], x_tiles, "fuA")

    h1 = layernorm(fuse1, "hln")
    f1A = load_panel("f1wT", "f1A", 0, 2, 2048)
    f1B = load_panel("f1wT", "f1B", 2, 4, 2048)
    f2A = load_panel("f2wT", "f2A", 0, 8, 512)
    f2B = load_panel("f2wT", "f2B", 8, 16, 512)
    fuse2 = ffn(f1A, f1B, f2A, f2B, "f1_b", "f2_b", h1, fuse1, "fuB")

    # LN2 on my tokens, exchange within the pair for full-token K/V
    h2loc = layernorm(fuse2, "h2l")
    for c in range(4):
        nc.sync.dma_start(t["cc_in"].ap()[c * 128:(c + 1) * 128, :], h2loc[c][:])
    nc.gpsimd.collective_compute(
        "AllGather", OP.bypass,
        replica_groups=[[0, 1], [2, 3], [4, 5], [6, 7]],
        ins=[t["cc_in"].ap().opt()],
        outs=[t["cc_out"].ap().opt()],
    )
    h2_slabs = []
    for s in range(2):
        slab = []
        for c in range(4):
            tag = f"x{c}" if s == 0 else f"hln{c}"
            st = S.tile([128, 512], F32R, tag=tag, name=f"h2s{s}{c}")
            nc.sync.dma_start(
                st[:],
                t["cc_out"].ap()[s * 512 + c * 128: s * 512 + (c + 1) * 128, :])
            slab.append(st)
        h2_slabs.append(slab)

    a2A = load_panel("a2w", "a2A", 0, 2, 2048)
    a2B = load_panel("a2w", "a2B", 2, 4, 2048)
    fuse3 = attention(a2A, a2B, "a2", h2loc, h2_slabs, t["w2T"], fuse2, "fuA")

    h3 = layernorm(fuse3, "hln")
    f3A = load_panel("f3wT", "f3A", 0, 2, 2048)
    f3B = load_panel("f3wT", "f3B", 2, 4, 2048)
    f4A = load_panel("f4wT", "f4A", 0, 8, 512)
    f4B = load_panel("f4wT", "f4B", 8, 16, 512)
    out_t = ffn(f3A, f3B, f4A, f4B, "f3_b", "f4_b", h3, fuse3, "fuB")

    for c in range(4):
        nc.sync.dma_start(t["fuseT"].ap()[c * 128:(c + 1) * 128, :],
                          out_t[c][:].bitcast(F32))

    P.release()
    S.release()


def _build():
    nc = bacc.Bacc("TRN2", target_bir_lowering=False, debug=False,
                   num_devices=NCORES)
    t = {}
    t["xT"] = nc.dram_tensor("xT", [SEQ, TQ], F32R, kind="ExternalInput")
    t["sT"] = nc.dram_tensor("sT", [STR, TK], F32R, kind="ExternalInput")
    t["pwT"] = nc.dram_tensor("pwT", [SEQ, STR], F32R, kind="ExternalInput")
    for nm in ("a1w", "a2w", "f1wT", "f3wT"):
        t[nm] = nc.dram_tensor(nm, [STR, 4 * STR], F32R, kind="ExternalInput")
    for nm in ("f2wT", "f4wT"):
        t[nm] = nc.dram_tensor(nm, [FF, STR], F32R, kind="ExternalInput")
    t["bias"] = nc.dram_tensor("bias", [128, NBCOL], F32, kind="ExternalInput")
    t["w1T"] = nc.dram_tensor("w1T", [H, TK, TQ], F32, kind="ExternalOutput")
    t["w2T"] = nc.dram_tensor("w2T", [H, TK, TQ], F32, kind="ExternalOutput")
    t["fuseT"] = nc.dram_tensor("fuseT", [STR, TQ], F32, kind="ExternalOutput")
    t["cc_in"] = nc.dram_tensor("cc_in", [STR, TQ], F32R)
    t["cc_out"] = nc.dram_tensor("cc_out", [2 * STR, TQ], F32R)

    with tile.TileContext(nc) as tc:
        _emit(tc, t)
    nc.compile()
    return nc


def _pack_bias(d):
    scale = DH ** -0.5
    tbl = np.zeros((128, NBCOL), np.float32)

    def put(name, vec):
        n = vec.shape[0] // 128
        tbl[:, BCOL[name]:BCOL[name] + n] = vec.reshape(n, 128).T

    put("p_b", d["p_b"])
    for pre in ("a1", "a2"):
        put(f"{pre}_qb", d[f"{pre}_qb"] * scale)
        put(f"{pre}_kb", d[f"{pre}_kb"])
        # fold v-bias through out projection: ob' = ob + vb @ ow.T
        put(f"{pre}_ob", d[f"{pre}_ob"] + d[f"{pre}_vb"] @ d[f"{pre}_ow"].T)
    for nm in ("f1_b", "f2_b", "f3_b", "f4_b"):
        put(nm, d[nm])
    return tbl


def kernel(**inputs):
    global LAST_RESULTS
    d = {k: np.ascontiguousarray(np.asarray(v, dtype=np.float32))
         for k, v in inputs.items()}
    scale = DH ** -0.5

    shared = {
        "pwT": np.ascontiguousarray(d["p_w"].T),
        "f1wT": np.ascontiguousarray(d["f1_w"].T),
        "f2wT": np.ascontiguousarray(d["f2_w"].T),
        "f3wT": np.ascontiguousarray(d["f3_w"].T),
        "f4wT": np.ascontiguousarray(d["f4_w"].T),
        "bias": _pack_bias(d),
    }
    for pre in ("a1", "a2"):
        shared[f"{pre}w"] = np.ascontiguousarray(np.concatenate(
            [(d[f"{pre}_qw"] * scale).T, d[f"{pre}_kw"].T,
             d[f"{pre}_vw"].T, d[f"{pre}_ow"].T], axis=1))

    in_maps = []
    for c in range(NCORES):
        b, p = c // 2, c % 2
        m = dict(shared)
        m["xT"] = np.ascontiguousarray(d["seq_embed"][b].T[:, p * TQ:(p + 1) * TQ])
        m["sT"] = np.ascontiguousarray(d["structure_embed"][b].T)
        in_maps.append(m)

    if TRACE:
        _install_ntff_hook()
    nc = _build()
    res = run_bass_kernel_spmd(
        nc, in_maps, core_ids=list(range(NCORES)),
        trace=TRACE,
        trace_cores=(list(range(NCORES)) if TRACE_ALL_CORES else [0]) if TRACE
        else None,
    )
    LAST_RESULTS = res

    fuse = np.empty((B, L, STR), np.float32)
    w1 = np.empty((B * H, L, TK), np.float32)
    w2 = np.empty((B * H, L, TK), np.float32)
    for c in range(NCORES):
        b, p = c // 2, c % 2
        r = res.results[c]
        qs = slice(p * TQ, (p + 1) * TQ)
        fuse[b, qs, :] = r["fuseT"].T
        w1[b * H:(b + 1) * H, qs, :] = r["w1T"].transpose(0, 2, 1)
        w2[b * H:(b + 1) * H, qs, :] = r["w2T"].transpose(0, 2, 1)
    return fuse, d["structure_embed"], w1, w2


# revision 17
# speedup vs baseline: 1.1382x; 1.0414x over previous
"""Trainium2 Bass kernel for a cross-modal transformer block (CrossModalBlock).

kernel(**inputs) takes the FULL unsharded inputs (numpy fp32) and returns the
full outputs (fuse, structure_embed, w1, w2) matching the reference.

Sharding (8 NeuronCores): data-parallel. Core c handles batch b=c//2, query
token half p=c%2 (512 of 1024 tokens). attn1 K/V come from structure_embed
(replicated per pair); attn2 K/V need post-LN2 activations of all 1024 tokens,
exchanged with a 1 MB pair-AllGather. No other collectives.

Device layout: feature-major activations [feature(partition), token(free)].
All matmuls run as float32r (full-rate). Attention scores are computed
transposed [k, q]; softmax denominators come from an extra ones-column in the
V operand of the probs@V matmul. Raw scores are written to DRAM as [h, k, q]
and transposed to [h, q, k] on the host during unsharding.
"""

import os
import sys

import numpy as np

sys.path.insert(0, "/opt/trn_rl_repo")
os.environ.setdefault("MYCRO_LOCAL_CACHE", "1")

import concourse.bass as bass  # noqa: E402
from concourse import bacc  # noqa: E402
import concourse.tile as tile  # noqa: E402
from concourse import mybir  # noqa: E402
from concourse.bass_utils import run_bass_kernel_spmd  # noqa: E402

B, L, SEQ, STR, H, DH, FF = 4, 1024, 1280, 512, 8, 64, 2048
EPS = 1e-5
NCORES = 8
TQ = 512          # query tokens per core
TK = 1024         # kv tokens (full batch)
F32 = mybir.dt.float32
F32R = mybir.dt.float32r
AF = mybir.ActivationFunctionType
OP = mybir.AluOpType

# bias table column offsets (each col holds 128 features)
BCOL = {}
_off = 0
for _name, _n in [
    ("p_b", 4), ("a1_qb", 4), ("a1_kb", 4), ("a1_ob", 4),
    ("f1_b", 16), ("f2_b", 4),
    ("a2_qb", 4), ("a2_kb", 4), ("a2_ob", 4),
    ("f3_b", 16), ("f4_b", 4),
]:
    BCOL[_name] = _off
    _off += _n
NBCOL = _off

TRACE = False
TRACE_ALL_CORES = False
LAST_RESULTS = None


def _install_ntff_hook():
    """This image's antenv lacks axon_hooks; synthesize it so trace=True works
    (mirrors trn_agent_boot._ntff_profile_via_ctypes). TRACE-only path."""
    import contextlib
    import ctypes
    import types

    try:
        from antenv.axon_hooks import get_axon_ntff_profile_hook  # noqa: F401
        return
    except ImportError:
        pass

    so_path = "/opt/axon/libaxon_pjrt.so"
    lib = ctypes.CDLL(so_path)
    if not hasattr(lib, "axon_start_nrt_profile"):
        hook = None
    else:
        lib.axon_start_nrt_profile.argtypes = [
            ctypes.POINTER(ctypes.c_int64), ctypes.c_size_t]
        lib.axon_start_nrt_profile.restype = ctypes.c_int64
        lib.axon_stop_nrt_profile.argtypes = [ctypes.c_char_p]
        lib.axon_stop_nrt_profile.restype = ctypes.c_int64

        @contextlib.contextmanager
        def hook(output_dir, device_ids):
            import jax

            jax.devices()
            if device_ids:
                ids = (ctypes.c_int64 * len(device_ids))(*device_ids)
                rc = lib.axon_start_nrt_profile(ids, len(device_ids))
            else:
                rc = lib.axon_start_nrt_profile(None, 0)
            if rc != 0:
                raise RuntimeError(f"axon_start_nrt_profile rc={rc}")
            try:
                yield
            finally:
                n = lib.axon_stop_nrt_profile(str(output_dir).encode())
                print(f"profile: {n} file(s) written to {output_dir}")

    mod = types.ModuleType("antenv.axon_hooks")
    mod.get_axon_ntff_profile_hook = lambda: hook
    mod.set_axon_ntff_profile_hook = lambda h: None
    sys.modules["antenv.axon_hooks"] = mod


def _emit(tc, t):
    nc = tc.nc

    S = tc.alloc_tile_pool(name="S", bufs=1)
    P = tc.alloc_tile_pool(name="P", bufs=2, space="PSUM")

    def mm(out, lhsT, rhs, start, stop):
        nc.tensor.matmul(out, lhsT.bitcast(F32R), rhs.bitcast(F32R),
                         start=start, stop=stop)

    # ---------------- constants ----------------
    onesf = S.tile([128, 8], F32, tag="onesf")
    nc.gpsimd.memset(onesf[:], 1.0)
    ones = S.tile([128, 1], F32R, tag="ones")
    nc.vector.tensor_copy(ones[:], onesf[:, 0:1])
    epsb = S.tile([1, 1], F32, tag="epsb")
    nc.gpsimd.memset(epsb[:], EPS)
    bias = S.tile([128, NBCOL], F32, tag="bias")
    nc.sync.dma_start(bias[:], t["bias"].ap())

    def bcol(name, i):
        return bias[:, BCOL[name] + i: BCOL[name] + i + 1]

    # ---------------- streamed weight/input panels (tag "w") ----------------
    def load_panel(name, nm, chunk_lo, chunk_hi, ncols):
        w = S.tile([128, (chunk_hi - chunk_lo) * ncols], F32R, tag="w", bufs=3,
                   name=nm)
        nc.sync.dma_start(
            w[:],
            t[name].ap().rearrange("(c p) n -> p c n", p=128)[:, chunk_lo:chunk_hi, :],
        )
        return w

    # =========================================================
    # proj_seq: x = seq @ p_w.T + p_b   (feature-major, my 512 tokens)
    # panel-outer accumulation, 4 psum halves in 2 "sc" tiles
    # =========================================================
    ps_proj = [P.tile([128, 1024], F32, tag="sc", name=f"psproj{i}")
               for i in range(2)]

    def proj_half(m):
        return ps_proj[m // 2][:, (m % 2) * 512:(m % 2) * 512 + 512]

    for p in range(2):
        pw_p = load_panel("pwT", f"pw{p}", 5 * p, 5 * p + 5, 512)
        x_p = load_panel("xT", f"xp{p}", 5 * p, 5 * p + 5, 512)
        for kcl in range(5):
            kc = 5 * p + kcl
            for m in range(4):
                mm(proj_half(m),
                   pw_p[:, kcl * 512 + m * 128: kcl * 512 + m * 128 + 128],
                   x_p[:, kcl * 512:(kcl + 1) * 512],
                   start=(kc == 0), stop=(kc == 9))
    x_tiles = []
    for m in range(4):
        xt = S.tile([128, 512], F32R, tag=f"x{m}", name=f"x{m}")
        nc.vector.tensor_scalar_add(xt[:], proj_half(m), bcol("p_b", m))
        x_tiles.append(xt)

    sT = S.tile([128, 4 * TK], F32R, tag="sT")  # structure.T [512,1024] resident
    nc.sync.dma_start(sT[:], t["sT"].ap().rearrange("(c p) n -> p c n", p=128))

    # =========================================================
    # attention (shared for attn1 / attn2)
    # =========================================================
    def attention(awA, awB, pre, q_src, kv_srcs, w_out, resid_tiles, out_tag):
        """q_src: 4 chunk APs [128, TQ]. kv_srcs: list of 2 slabs, each 4 chunk
        APs [128, 512]. Returns 4 tiles [128, TQ] = resid + attn_out + ob'."""

        def aw(kc, off, width=128):
            w = awA if kc < 2 else awB
            return w[:, (kc % 2) * 2048 + off: (kc % 2) * 2048 + off + width]

        # ---- q projection (+qb') ----
        q_tiles = []
        for m in range(4):
            ps = P.tile([128, 512], F32, tag="mm", name=f"{pre}qp{m}")
            for kc in range(4):
                mm(ps[:], aw(kc, m * 128), q_src[kc][:, :],
                   start=(kc == 0), stop=(kc == 3))
            qt = S.tile([128, 512], F32R, tag=f"atq{m}", name=f"{pre}q{m}")
            nc.vector.tensor_scalar_add(qt[:], ps[:], bcol(f"{pre}_qb", m))
            q_tiles.append(qt)

        # ---- k projection (+kb), all TK tokens ----
        k_tiles = []
        for m in range(4):
            kt = S.tile([128, TK], F32R, tag=f"atk{m}", name=f"{pre}k{m}")
            for s in range(2):
                ps = P.tile([128, 512], F32, tag="mm", name=f"{pre}kp{m}{s}")
                for kc in range(4):
                    mm(ps[:], aw(kc, 512 + m * 128), kv_srcs[s][kc][:, :],
                       start=(kc == 0), stop=(kc == 3))
                nc.vector.tensor_scalar_add(
                    kt[:, s * 512:(s + 1) * 512], ps[:], bcol(f"{pre}_kb", m))
            k_tiles.append(kt)

        # ---- v projection, token-major [tok, feat] + ones columns ----
        v_tiles = []
        for s in range(2):
            for tt in range(4):
                i = s * 4 + tt
                ps = P.tile([128, 512], F32, tag="mm", name=f"{pre}vp{i}")
                for kc in range(4):
                    mm(ps[:], kv_srcs[s][kc][:, tt * 128: tt * 128 + 128],
                       aw(kc, 1024, 512), start=(kc == 0), stop=(kc == 3))
                vt = S.tile([128, 8 * 65], F32R, tag=f"atv{i}", name=f"{pre}v{i}")
                v3 = vt.rearrange("p (h c) -> p h c", c=65)
                nc.vector.tensor_copy(
                    v3[:, :, 64:65],
                    onesf[:, 0:8].rearrange("p (a b) -> p a b", b=1))
                nc.vector.tensor_copy(
                    v3[:, :, 0:64], ps.rearrange("p (h c) -> p h c", c=64))
                v_tiles.append(vt)

        # ---- per head: scores^T -> raw dma + exp -> probs@V -> normalize ----
        ctx_tiles = [S.tile([128, 512], F32R, tag=f"atcf{i}", name=f"{pre}cf{i}")
                     for i in range(4)]
        for h in range(H):
            pt, off = h // 2, 64 * (h & 1)
            q_h = q_tiles[pt][off:off + 64, :]
            cps = P.tile([128, 512], F32, tag="cx", name=f"{pre}cps{h}")
            for j in range(4):
                sps = P.tile([128, 1024], F32, tag="sc", name=f"{pre}sps{h}{j}")
                for j2 in range(2):
                    ki = 2 * j + j2
                    mm(sps[:, j2 * 512:(j2 + 1) * 512],
                       k_tiles[pt][off:off + 64, ki * 128: ki * 128 + 128],
                       q_h, start=True, stop=True)
                dst = w_out.ap()[h, j * 256:(j + 1) * 256, :].rearrange(
                    "(j2 kp) q -> kp j2 q", kp=128)
                raw = S.tile([128, 1024], F32, tag="raw", bufs=2,
                             name=f"{pre}w{h}{j}")
                if j % 2 == 0:
                    nc.vector.tensor_copy(raw[:], sps[:, :])
                else:
                    nc.scalar.activation(raw[:], sps[:, :], AF.Identity)
                nc.sync.dma_start(dst, raw.rearrange("p (j2 q) -> p j2 q", j2=2))
                et = S.tile([128, 1024], F32R, tag="expT", bufs=2,
                            name=f"{pre}e{h}{j}")
                nc.scalar.activation(et[:], sps[:], AF.Exp)
                for j2 in range(2):
                    ki = 2 * j + j2
                    mm(cps[0:65, :], v_tiles[ki][:, h * 65: h * 65 + 65],
                       et[:, j2 * 512:(j2 + 1) * 512],
                       start=(ki == 0), stop=(ki == 7))
            den = S.tile([1, 512], F32, tag="sm", bufs=7, name=f"{pre}d{h}")
            nc.vector.tensor_copy(den[:], cps[64:65, :])
            rec = S.tile([1, 512], F32, tag="sm", bufs=7, name=f"{pre}r{h}")
            rsc = S.tile([1, 512], F32, tag="rsc", bufs=2, name=f"{pre}rs{h}")
            nc.vector.reciprocal_approx_accurate(rec[:], den[:], rsc[:])
            rbc = S.tile([64, 512], F32, tag="rbc", bufs=2, name=f"{pre}rb{h}")
            nc.gpsimd.partition_broadcast(rbc[:], rec[:])
            nc.vector.tensor_mul(ctx_tiles[pt][off:off + 64, :],
                                 cps[0:64, :], rbc[:])

        # ---- out projection + ob' + residual ----
        out_tiles = []
        for m in range(4):
            ps = P.tile([128, 512], F32, tag="mm", name=f"{pre}op{m}")
            for kc in range(4):
                mm(ps[:], aw(kc, 1536 + m * 128), ctx_tiles[kc][:, :],
                   start=(kc == 0), stop=(kc == 3))
            ot = S.tile([128, 512], F32R, tag=f"{out_tag}{m}", name=f"{pre}o{m}")
            nc.vector.scalar_tensor_tensor(
                ot[:], in0=ps[:], scalar=bcol(f"{pre}_ob", m),
                in1=resid_tiles[m][:, :].bitcast(F32), op0=OP.add, op1=OP.add)
            out_tiles.append(ot)
        return out_tiles

    # =========================================================
    # layernorm (feature-major over STR=512; g=1, b=0 in this model)
    # =========================================================
    def layernorm(x_tiles, out_tag):
        s1 = P.tile([128, 512], F32, tag="cx", name=f"{out_tag}s1")
        s2 = P.tile([128, 512], F32, tag="cx", name=f"{out_tag}s2")
        sq_tiles = []
        for c in range(4):
            sqt = S.tile([128, 512], F32R, tag="sq", bufs=2, name=f"{out_tag}sq{c}")
            xf = x_tiles[c][:, :].bitcast(F32)
            nc.vector.tensor_mul(sqt[:], xf, xf)
            sq_tiles.append(sqt)
        for c in range(4):
            mm(s1[0:1, :], ones[:, 0:1], x_tiles[c][:, :],
               start=(c == 0), stop=(c == 3))
        for c in range(4):
            mm(s2[0:1, :], ones[:, 0:1], sq_tiles[c][:],
               start=(c == 0), stop=(c == 3))

        def sm(nm):
            return S.tile([1, 512], F32, tag="sm", bufs=7, name=f"{out_tag}{nm}")

        mean = sm("mean")
        nc.vector.tensor_scalar_mul(mean[:], s1[0:1, :], 1.0 / STR)
        ms = sm("ms")
        nc.vector.tensor_scalar_mul(ms[:], s2[0:1, :], 1.0 / STR)
        m2 = sm("m2")
        nc.vector.tensor_mul(m2[:], mean[:], mean[:])
        var = sm("var")
        nc.vector.tensor_sub(var[:], ms[:], m2[:])
        std = sm("std")
        nc.scalar.activation(std[:], var[:], AF.Sqrt, bias=epsb[:], scale=1.0)
        rstd = sm("rstd")
        rsc2 = S.tile([1, 512], F32, tag="rsc", bufs=2, name=f"{out_tag}rsc")
        nc.vector.reciprocal_approx_accurate(rstd[:], std[:], rsc2[:])
        mr = sm("mr")
        nc.vector.tensor_mul(mr[:], mean[:], rstd[:])
        rstd_bc = S.tile([128, 512], F32, tag="bc", bufs=2, name=f"{out_tag}rb")
        nc.gpsimd.partition_broadcast(rstd_bc[:], rstd[:])
        mr_bc = S.tile([128, 512], F32, tag="bc", bufs=2, name=f"{out_tag}mb")
        nc.gpsimd.partition_broadcast(mr_bc[:], mr[:])
        out = []
        for c in range(4):
            otag = f"atcf{c}" if out_tag == "h2l" else f"{out_tag}{c}"
            o = S.tile([128, 512], F32R, tag=otag, name=f"{out_tag}{c}")
            nc.vector.tensor_mul(o[:], x_tiles[c][:, :].bitcast(F32), rstd_bc[:])
            nc.vector.tensor_sub(o[:], o[:].bitcast(F32), mr_bc[:])
            out.append(o)
        return out

    # =========================================================
    # ffn: resid + gelu(x@f1+b1)@f2 + b2
    # =========================================================
    def ffn(f1A, f1B, f2A, f2B, b1name, b2name, x_tiles, resid_tiles, out_tag):
        acc = [P.tile([128, 512], F32, tag=("mm" if m < 2 else "cx"),
                      name=f"{out_tag}acc{m}") for m in range(4)]
        h_back = []
        for hc in range(16):
            ps = P.tile([128, 512], F32, tag="sc", name=f"{out_tag}hp{hc}")
            for kc in range(4):
                w = f1A if kc < 2 else f1B
                mm(ps[:], w[:, (kc % 2) * 2048 + hc * 128: (kc % 2) * 2048 + hc * 128 + 128],
                   x_tiles[kc][:, :], start=(kc == 0), stop=(kc == 3))
            ht = S.tile([128, 512], F32R, tag=f"ffnh{hc % 8}", name=f"{out_tag}h{hc}")
            nc.scalar.activation(ht[:], ps[:], AF.Gelu, bias=bcol(b1name, hc),
                                 scale=1.0)
            h_back.append(ht)
            if hc < 8:
                for m in range(4):
                    mm(acc[m][:], f2A[:, hc * 512 + m * 128: hc * 512 + m * 128 + 128],
                       ht[:], start=(hc == 0), stop=False)
        # second half of the down-projection after mm1 fully drains f1A/f1B
        # (f2B's panel slot evicts f1A; issuing these earlier would deadlock PE)
        for hc in range(8, 16):
            ht = h_back[hc]
            for m in range(4):
                mm(acc[m][:], f2B[:, (hc - 8) * 512 + m * 128: (hc - 8) * 512 + m * 128 + 128],
                   ht[:], start=False, stop=(hc == 15))
        out_tiles = []
        for m in range(4):
            ot = S.tile([128, 512], F32R, tag=f"{out_tag}{m}", name=f"{out_tag}o{m}")
            nc.vector.scalar_tensor_tensor(
                ot[:], in0=acc[m][:], scalar=bcol(b2name, m),
                in1=resid_tiles[m][:, :].bitcast(F32), op0=OP.add, op1=OP.add)
            out_tiles.append(ot)
        return out_tiles

    # =========================================================
    # the block
    # =========================================================
    a1A = load_panel("a1w", "a1A", 0, 2, 2048)
    a1B = load_panel("a1w", "a1B", 2, 4, 2048)
    s_slabs = [[sT[:, c * TK + s * 512: c * TK + (s + 1) * 512] for c in range(4)]
               for s in range(2)]
    fuse1 = attention(a1A, a1B, "a1", x_tiles, s_slabs, t["w1T"], x_tiles, "fuA")

    h1 = layernorm(fuse1, "hln")
    f1A = load_panel("f1wT", "f1A", 0, 2, 2048)
    f1B = load_panel("f1wT", "f1B", 2, 4, 2048)
    f2A = load_panel("f2wT", "f2A", 0, 8, 512)
    f2B = load_panel("f2wT", "f2B", 8, 16, 512)
    fuse2 = ffn(f1A, f1B, f2A, f2B, "f1_b", "f2_b", h1, fuse1, "fuB")

    # LN2 on my tokens, exchange within the pair for full-token K/V
    a2A = load_panel("a2w", "a2A", 0, 2, 2048)
    a2B = load_panel("a2w", "a2B", 2, 4, 2048)
    h2loc = layernorm(fuse2, "h2l")
    for c in range(4):
        nc.sync.dma_start(t["cc_in"].ap()[c * 128:(c + 1) * 128, :], h2loc[c][:])
    nc.gpsimd.collective_compute(
        "AllGather", OP.bypass,
        replica_groups=[[0, 1], [2, 3], [4, 5], [6, 7]],
        ins=[t["cc_in"].ap().opt()],
        outs=[t["cc_out"].ap().opt()],
    )
    h2_slabs = []
    for s in range(2):
        slab = []
        for c in range(4):
            tag = f"x{c}" if s == 0 else f"hln{c}"
            st = S.tile([128, 512], F32R, tag=tag, name=f"h2s{s}{c}")
            nc.sync.dma_start(
                st[:],
                t["cc_out"].ap()[s * 512 + c * 128: s * 512 + (c + 1) * 128, :])
            slab.append(st)
        h2_slabs.append(slab)

    fuse3 = attention(a2A, a2B, "a2", h2loc, h2_slabs, t["w2T"], fuse2, "fuA")

    h3 = layernorm(fuse3, "hln")
    f3A = load_panel("f3wT", "f3A", 0, 2, 2048)
    f3B = load_panel("f3wT", "f3B", 2, 4, 2048)
    f4A = load_panel("f4wT", "f4A", 0, 8, 512)
    f4B = load_panel("f4wT", "f4B", 8, 16, 512)
    out_t = ffn(f3A, f3B, f4A, f4B, "f3_b", "f4_b", h3, fuse3, "fuB")

    for c in range(4):
        nc.sync.dma_start(t["fuseT"].ap()[c * 128:(c + 1) * 128, :],
                          out_t[c][:].bitcast(F32))

    P.release()
    S.release()


def _build():
    nc = bacc.Bacc("TRN2", target_bir_lowering=False, debug=False,
                   num_devices=NCORES)
    t = {}
    t["xT"] = nc.dram_tensor("xT", [SEQ, TQ], F32R, kind="ExternalInput")
    t["sT"] = nc.dram_tensor("sT", [STR, TK], F32R, kind="ExternalInput")
    t["pwT"] = nc.dram_tensor("pwT", [SEQ, STR], F32R, kind="ExternalInput")
    for nm in ("a1w", "a2w", "f1wT", "f3wT"):
        t[nm] = nc.dram_tensor(nm, [STR, 4 * STR], F32R, kind="ExternalInput")
    for nm in ("f2wT", "f4wT"):
        t[nm] = nc.dram_tensor(nm, [FF, STR], F32R, kind="ExternalInput")
    t["bias"] = nc.dram_tensor("bias", [128, NBCOL], F32, kind="ExternalInput")
    t["w1T"] = nc.dram_tensor("w1T", [H, TK, TQ], F32, kind="ExternalOutput")
    t["w2T"] = nc.dram_tensor("w2T", [H, TK, TQ], F32, kind="ExternalOutput")
    t["fuseT"] = nc.dram_tensor("fuseT", [STR, TQ], F32, kind="ExternalOutput")
    t["cc_in"] = nc.dram_tensor("cc_in", [STR, TQ], F32R)
    t["cc_out"] = nc.dram_tensor("cc_out", [2 * STR, TQ], F32R)

    with tile.TileContext(nc) as tc:
        _emit(tc, t)
    nc.compile()
    return nc


def _pack_bias(d):
    scale = DH ** -0.5
    tbl = np.zeros((128, NBCOL), np.float32)

    def put(name, vec):
        n = vec.shape[0] // 128
        tbl[:, BCOL[name]:BCOL[name] + n] = vec.reshape(n, 128).T

    put("p_b", d["p_b"])
    for pre in ("a1", "a2"):
        put(f"{pre}_qb", d[f"{pre}_qb"] * scale)
        put(f"{pre}_kb", d[f"{pre}_kb"])
        # fold v-bias through out projection: ob' = ob + vb @ ow.T
        put(f"{pre}_ob", d[f"{pre}_ob"] + d[f"{pre}_vb"] @ d[f"{pre}_ow"].T)
    for nm in ("f1_b", "f2_b", "f3_b", "f4_b"):
        put(nm, d[nm])
    return tbl


def kernel(**inputs):
    global LAST_RESULTS
    d = {k: np.ascontiguousarray(np.asarray(v, dtype=np.float32))
         for k, v in inputs.items()}
    scale = DH ** -0.5

    shared = {
        "pwT": np.ascontiguousarray(d["p_w"].T),
        "f1wT": np.ascontiguousarray(d["f1_w"].T),
        "f2wT": np.ascontiguousarray(d["f2_w"].T),
        "f3wT": np.ascontiguousarray(d["f3_w"].T),
        "f4wT": np.ascontiguousarray(d["f4_w"].T),
        "bias": _pack_bias(d),
    }
    for pre in ("a1", "a2"):
        shared[f"{pre}w"] = np.ascontiguousarray(np.concatenate(
            [(d[f"{pre}_qw"] * scale).T, d[f"{pre}_kw"].T,
             d[f"{pre}_vw"].T, d[f"{pre}_ow"].T], axis=1))

    in_maps = []
    for c in range(NCORES):
        b, p = c // 2, c % 2
        m = dict(shared)
        m["xT"] = np.ascontiguousarray(d["seq_embed"][b].T[:, p * TQ:(p + 1) * TQ])
        m["sT"] = np.ascontiguousarray(d["structure_embed"][b].T)
        in_maps.append(m)

    if TRACE:
        _install_ntff_hook()
    nc = _build()
    res = run_bass_kernel_spmd(
        nc, in_maps, core_ids=list(range(NCORES)),
        trace=TRACE,
        trace_cores=(list(range(NCORES)) if TRACE_ALL_CORES else [0]) if TRACE
        else None,
    )
    LAST_RESULTS = res

    fuse = np.empty((B, L, STR), np.float32)
    w1 = np.empty((B * H, L, TK), np.float32)
    w2 = np.empty((B * H, L, TK), np.float32)
    for c in range(NCORES):
        b, p = c // 2, c % 2
        r = res.results[c]
        qs = slice(p * TQ, (p + 1) * TQ)
        fuse[b, qs, :] = r["fuseT"].T
        w1[b * H:(b + 1) * H, qs, :] = r["w1T"].transpose(0, 2, 1)
        w2[b * H:(b + 1) * H, qs, :] = r["w2T"].transpose(0, 2, 1)
    return fuse, d["structure_embed"], w1, w2
